# revision 25
# baseline (speedup 1.0000x reference)
"""DPOT2D layer (AFNO-style) Trainium2 kernel.

out = x + irfft2_pad(blockMLP(trunc64(rfft2(x))))   (ortho norm)

Sharding: tensor-parallel over the 8 block-diagonal channel groups — core n
gets channels [n*64, (n+1)*64) and its block's MLP weights. Blocks never mix,
so there is zero cross-core communication.

Per core, every FFT stage is a DFT matmul on the TensorEngine (bf16 operands,
fp32 PSUM accumulation), with PE-transpose corner turns between stages:

  A:  U[k1s,(w,c)]   = F_h^T  @ x          (contract h, 2x128 K-chunks)
  t1: V[w,(c,k1s)]   = corner turn of U
  B:  Y[k2s,(c,k1)]  = DFT_w on complex U  (re/im column accumulation)
  t2: Yt[c,(k1,k2s)] = corner turn of Y
  L1: o1 = gelu(W1 Y + b1)                 (K=64, re/im col accumulation)
  L2: O2[(o2r|o2i),(k1,k2)] = W2 o1 + b2   (K=128)
  t3: R[k2,(k1,o2s)] = corner turn of O2
  iW: G[w,(k1,c')]   = hermitian irfft_w matmuls (re/im col accumulation)
  t4: Ght[k1s,(w,c')]= corner turn of G (re/im interleaved -> k1-stack)
  iH: x'[h,(w,c')]   = E_h^T @ Ght, + residual x (fp32), DMA out
"""

import numpy as np
import ml_dtypes

import concourse.bass as bass
import concourse.mybir as mybir
from concourse import bacc
from concourse import masks
from concourse.tile import TileContext
from concourse.bass_utils import run_bass_kernel_spmd

B = 2
H = 256
W = 256
C = 512
NB = 8
BS = 64          # channels per block (= per core)
KEEP = 64        # kept modes per spatial dim
HID = 128
P = 128

BF16 = mybir.dt.bfloat16
F32 = mybir.dt.float32
AF = mybir.ActivationFunctionType

_CACHED_NC = None


def _host_consts():
    """DFT matrices shared by all cores (fp32 -> bf16)."""
    h = np.arange(H, dtype=np.float64)[:, None]
    k = np.arange(KEEP, dtype=np.float64)[None, :]
    th = 2.0 * np.pi * h * k / H
    F = np.concatenate([np.cos(th), -np.sin(th)], axis=1) / 16.0      # (256,128)
    Fwre, Fwim = F[:, :KEEP], F[:, KEEP:]
    lb_re = np.concatenate([Fwre, Fwim], axis=1)                      # (256,128)
    lb_im = np.concatenate([-Fwim, Fwre], axis=1)
    alpha = np.where(np.arange(KEEP) == 0, 1.0, 2.0)
    k2 = np.arange(KEEP, dtype=np.float64)[:, None]
    wv = np.arange(W, dtype=np.float64)[None, :]
    tw = 2.0 * np.pi * k2 * wv / W
    Ca = alpha[:, None] * np.cos(tw) / 16.0                           # (64,256)
    Sa = alpha[:, None] * np.sin(tw) / 16.0
    k1 = np.arange(KEEP, dtype=np.float64)[:, None]
    hv = np.arange(H, dtype=np.float64)[None, :]
    tih = 2.0 * np.pi * k1 * hv / H
    Ehc = np.cos(tih) / 16.0                                          # (64,256)
    Ehs = np.sin(tih) / 16.0
    lih_full = np.concatenate([Ehc, -Ehs], axis=0)                    # (128,256)

    bf = ml_dtypes.bfloat16
    ffwd = [F[0:128], F[128:256]]
    lbw = [[lb_re[0:128], lb_im[0:128]], [lb_re[128:256], lb_im[128:256]]]
    liw = [[[Ca[:, 0:128], -Sa[:, 0:128]], [Sa[:, 0:128], Ca[:, 0:128]]],
           [[Ca[:, 128:256], -Sa[:, 128:256]], [Sa[:, 128:256], Ca[:, 128:256]]]]
    lih = [lih_full[:, 0:128], lih_full[:, 128:256]]
    return ffwd, lbw, liw, lih


def _build_nc(loop_iters=0, probe=None):
    """loop_iters>0 wraps the whole per-batch pipeline in an on-device
    For_i repeat loop — used only by the timing harness to amortize the
    ~80ms axon dispatch overhead out of the measurement.
    probe: None | 'dma' (DMAs only) | 'compute' (no input/residual DMAs)."""
    nc = bacc.Bacc()

    xbf = nc.declare_dram_parameter("xbf", [B, H, W, BS], BF16, isOutput=False)
    cb128_d = nc.declare_dram_parameter("cb128", [P, 10 * P], BF16, isOutput=False)
    cb64_d = nc.declare_dram_parameter("cb64", [KEEP, 12 * P], BF16, isOutput=False)
    cbias_d = nc.declare_dram_parameter("cbias", [P, 3], F32, isOutput=False)
    out = nc.declare_dram_parameter("out", [B, H, W, BS], F32, isOutput=True)

    with TileContext(nc) as tc:
        consts = tc.alloc_tile_pool(name="consts", bufs=1)
        ident = consts.tile([P, P], BF16, name="ident")
        masks.make_identity(nc, ident[:])

        cb128 = consts.tile([P, 10 * P], BF16, name="cb128")
        nc.sync.dma_start(out=cb128[:], in_=cb128_d[:])
        cb64 = consts.tile([KEEP, 12 * P], BF16, name="cb64")
        nc.sync.dma_start(out=cb64[:], in_=cb64_d[:])
        cbias = consts.tile([P, 3], F32, name="cbias")
        nc.sync.dma_start(out=cbias[:], in_=cbias_d[:])

        def s128(i):
            return cb128[:, i * P:(i + 1) * P]

        def s64(i):
            return cb64[:, i * P:(i + 1) * P]

        # layout: fw0 fw1 lbw00 lbw01 lbw10 lbw11 m2_0 m2_1 lih0 lih1
        FW = [s128(0), s128(1)]
        LBW = [[s128(2), s128(3)], [s128(4), s128(5)]]
        M2 = [s128(6), s128(7)]
        LIH = [s128(8), s128(9)]
        # layout: m1_00 m1_01 m1_10 m1_11 liw000 liw001 liw010 liw011
        #         liw100 liw101 liw110 liw111
        M1 = [[s64(0), s64(1)], [s64(2), s64(3)]]
        LIW = [[[s64(4 + 4 * wh + 2 * j + ss) for ss in range(2)]
                for j in range(2)] for wh in range(2)]
        b1s_t = [cbias[:, 0:1], cbias[:, 1:2]]
        b2s_t = cbias[:, 2:3]

        # copy-engine rotation (PSUM-capable engines only: DVE + ACT)
        cp_cnt = [0]

        _rot = [0, 1, 0, 1, 0]   # 3 DVE : 2 ACT

        def cp(dst, src):
            i = _rot[cp_cnt[0] % 5]
            cp_cnt[0] += 1
            if i == 0:
                nc.vector.tensor_copy(out=dst, in_=src)
            else:
                nc.scalar.activation(out=dst, in_=src, func=AF.Copy)

        # Tag-sharing across stage lifetimes keeps SBUF within budget:
        #   tagA/tagB: U[wh] -> G[wh]   tagC/tagD: V[wh] -> Ght[wh]
        #   tagE: Y -> R                tagF: Yt -> O2
        sb = tc.alloc_tile_pool(name="sb", bufs=1)
        xin = tc.alloc_tile_pool(name="xin", bufs=2)
        xres_p = tc.alloc_tile_pool(name="xres", bufs=1)
        outp = tc.alloc_tile_pool(name="outp", bufs=2)
        pmm = tc.alloc_tile_pool(name="pmm", bufs=4, space="PSUM")
        ptp = tc.alloc_tile_pool(name="ptp", bufs=4, space="PSUM")

        import contextlib
        loop_ctx = tc.For_i(0, loop_iters, 1) if loop_iters else contextlib.nullcontext()
        with loop_ctx:
            if probe == "dma":
                _emit_dma_probe(nc, tc, locals())
            else:
                _emit_body(nc, tc, locals(), skip_dma=(probe == "compute"))
        ptp.release()
        pmm.release()
        outp.release()
        xres_p.release()
        xin.release()
        sb.release()
        consts.release()
    nc.compile()
    return nc


def _emit_dma_probe(nc, tc, env):
    """Same DMA traffic as the real kernel (x-in bf16, x-res f32, out f32),
    no compute: out tiles are fed straight from the xres tiles."""
    xbf = env["xbf"]; out = env["out"]
    xin = env["xin"]; xres_p = env["xres_p"]
    for b in range(B):
        for wc in range(8):
            for hh in range(2):
                t = xin.tile([P, 32, BS], BF16, tag=f"xin{hh}",
                             name=f"pxin{hh}_{b}_{wc}")
                nc.sync.dma_start(
                    out=t[:],
                    in_=xbf[b, hh * P:(hh + 1) * P, wc * 32:(wc + 1) * 32, :])
        for hc in range(2):
            for q8 in range(8):
                xr = xres_p.tile([P, 32, BS], BF16, tag="xres",
                                 name=f"pxr_{b}_{hc}_{q8}")
                nc.sync.dma_start(
                    out=xr[:],
                    in_=xbf[b, hc * P:(hc + 1) * P, q8 * 32:(q8 + 1) * 32, :])
                nc.sync.dma_start(
                    out=out[b, hc * P:(hc + 1) * P, q8 * 32:(q8 + 1) * 32, :],
                    in_=xr[:])


def _emit_body(nc, tc, env, skip_dma=False):
    xbf = env["xbf"]; out = env["out"]
    FW = env["FW"]; LBW = env["LBW"]; M1 = env["M1"]; M2 = env["M2"]
    LIW = env["LIW"]; LIH = env["LIH"]; b1s_t = env["b1s_t"]; b2s_t = env["b2s_t"]
    ident = env["ident"]; cp = env["cp"]; cp_cnt = env["cp_cnt"]
    sb = env["sb"]; xin = env["xin"]; xres_p = env["xres_p"]; outp = env["outp"]
    pmm = env["pmm"]; ptp = env["ptp"]

    if True:
        for b in range(B):
            # ------- fused stage A: V[wh][w, (c, k1s)] = (x-chunk)^T @ F_h -----
            # lhsT = x tile (h, w-chunk) at fixed c, rhs = F-stack (h, k1s):
            # out psum (w 128, k1s 128), accumulated over the two h-halves.
            V = [sb.tile([P, BS, P], BF16, tag=f"tagCD{wh}", name=f"V{wh}_{b}")
                 for wh in range(2)]
            for wcl in range(2):         # w chunks of 128
                xt = []
                for hh in range(2):
                    t = xin.tile([P, P, BS], BF16, tag=f"xin{hh}",
                                 name=f"xin{hh}_{b}_{wcl}")
                    if not skip_dma:
                        nc.sync.dma_start(
                            out=t[:],
                            in_=xbf[b, hh * P:(hh + 1) * P, wcl * P:(wcl + 1) * P, :])
                    else:
                        nc.sync.dma_start(
                            out=t[0:1, 0:1, :],
                            in_=xbf[b, 0:1, 0:1, :])
                    xt.append(t)
                for c in range(BS):
                    ps = pmm.tile([P, P], F32, tag="mm", name=f"psA_{b}_{wcl}_{c}")
                    nc.tensor.matmul(ps[:], xt[0][:, :, c], FW[0],
                                     start=True, stop=False)
                    nc.tensor.matmul(ps[:], xt[1][:, :, c], FW[1],
                                     start=False, stop=True)
                    cp(V[wcl][:, c, :], ps[:])

            # ---------------- stage B: Y (128=k2s, (c 64, k1 64)) --------------
            Y = sb.tile([P, BS, KEEP], BF16, tag="tagE", name=f"Y_{b}")
            for nn in range(8):          # 8 c per chunk -> N=512
                ps = pmm.tile([P, 8, KEEP], F32, tag="mm", name=f"psB_{b}_{nn}")
                first = True
                for wh in range(2):
                    for s in range(2):   # 0: re cols (k1s 0:64), 1: im cols
                        rhs = V[wh][:, nn * 8:(nn + 1) * 8, s * KEEP:(s + 1) * KEEP]
                        nc.tensor.matmul(ps[:], LBW[wh][s], rhs,
                                         start=first, stop=(wh == 1 and s == 1))
                        first = False
                cp(Y[:, nn * 8:(nn + 1) * 8, :], ps[:])

            # ---------------- turn2: Yt (64=c, (k1 64, k2s 128)) ---------------
            Yt = sb.tile([BS, KEEP, P], BF16, tag="tagF", name=f"Yt_{b}")
            for k1 in range(KEEP):
                pt = ptp.tile([BS, P], BF16, tag="tp", name=f"t2_{b}_{k1}")
                nc.tensor.transpose(pt[:], Y[:, :, k1], ident[:])
                cp(Yt[:, k1, :], pt[:])

            # ---------------- MLP L1 (K=64) + gelu -----------------------------
            o1 = [sb.tile([HID, KEEP, KEEP], BF16, tag=f"o1_{j}", name=f"o1_{j}_{b}")
                  for j in range(2)]
            for j in range(2):
                for nn in range(8):      # 8 k1 per chunk -> N=512
                    ps = pmm.tile([HID, 8, KEEP], F32, tag="mm",
                                  name=f"ps1_{b}_{j}_{nn}")
                    nc.tensor.matmul(
                        ps[:], M1[j][0],
                        Yt[:, nn * 8:(nn + 1) * 8, 0:KEEP], start=True, stop=False)
                    nc.tensor.matmul(
                        ps[:], M1[j][1],
                        Yt[:, nn * 8:(nn + 1) * 8, KEEP:P], start=False, stop=True)
                    nc.scalar.activation(out=o1[j][:, nn * 8:(nn + 1) * 8, :],
                                         in_=ps[:], func=AF.Gelu, bias=b1s_t[j])

            # ---------------- MLP L2 (K=128) + bias ----------------------------
            O2 = sb.tile([P, KEEP, KEEP], BF16, tag="tagF", name=f"O2_{b}")
            for nn in range(8):
                ps = pmm.tile([P, 8, KEEP], F32, tag="mm", name=f"ps2_{b}_{nn}")
                nc.tensor.matmul(ps[:], M2[0], o1[0][:, nn * 8:(nn + 1) * 8, :],
                                 start=True, stop=False)
                nc.tensor.matmul(ps[:], M2[1], o1[1][:, nn * 8:(nn + 1) * 8, :],
                                 start=False, stop=True)
                if cp_cnt[0] % 2 == 0:
                    nc.vector.tensor_scalar_add(
                        out=O2[:, nn * 8:(nn + 1) * 8, :], in0=ps[:], scalar1=b2s_t)
                else:
                    nc.scalar.activation(out=O2[:, nn * 8:(nn + 1) * 8, :],
                                         in_=ps[:], func=AF.Identity, bias=b2s_t)
                cp_cnt[0] += 1

            # ---------------- turn3: R (64=k2, (k1 64, o2s 128)) ---------------
            R = sb.tile([KEEP, KEEP, P], BF16, tag="tagE", name=f"R_{b}")
            for k1 in range(KEEP):
                pt = ptp.tile([KEEP, P], BF16, tag="tp", name=f"t3_{b}_{k1}")
                nc.tensor.transpose(pt[:], O2[:, k1, :], ident[:])
                cp(R[:, k1, :], pt[:])

            # ---------------- invW: G[wh] (128=w, (j 2, k1 64, c' 64)) ---------
            G = [sb.tile([P, 2, KEEP, BS], BF16, tag=f"tagAB{wh}", name=f"G{wh}_{b}")
                 for wh in range(2)]
            for wh in range(2):
                for j in range(2):       # 0: Gre, 1: Gim
                    for nn in range(8):  # 8 k1 per chunk
                        ps = pmm.tile([P, 8, BS], F32, tag="mm",
                                      name=f"psW_{b}_{wh}_{j}_{nn}")
                        nc.tensor.matmul(
                            ps[:], LIW[wh][j][0],
                            R[:, nn * 8:(nn + 1) * 8, 0:KEEP],
                            start=True, stop=False)
                        nc.tensor.matmul(
                            ps[:], LIW[wh][j][1],
                            R[:, nn * 8:(nn + 1) * 8, KEEP:P],
                            start=False, stop=True)
                        cp(G[wh][:, j, nn * 8:(nn + 1) * 8, :], ps[:])

            # ---------------- turn4: Ght (128=k1s, (w 256, c' 64)) -------------
            Ght = [sb.tile([P, P, BS], BF16, tag=f"tagCD{wh}", name=f"Ght{wh}_{b}")
                   for wh in range(2)]
            for wh in range(2):
                for c in range(BS):
                    pt = ptp.tile([P, P], BF16, tag="tp", name=f"t4_{b}_{wh}_{c}")
                    # free slice (j 2, k1 64) -> out partitions [k1re | k1im]
                    nc.tensor.transpose(pt[:], G[wh][:, :, :, c], ident[:])
                    cp(Ght[wh][:, :, c], pt[:])

            # ---------------- invH + residual + store --------------------------
            for hc in range(2):
                for q4 in range(4):      # groups of 64 w
                    xr = xres_p.tile([P, 64, BS], BF16, tag="xres",
                                     name=f"xr_{b}_{hc}_{q4}")
                    if not skip_dma:
                        nc.sync.dma_start(
                            out=xr[:],
                            in_=xbf[b, hc * P:(hc + 1) * P, q4 * 64:(q4 + 1) * 64, :])
                    else:
                        nc.sync.dma_start(
                            out=xr[0:1, 0:1, :], in_=xbf[b, 0:1, 0:1, :])
                    for g in range(2):   # out groups of 32 w
                        ot = outp.tile([P, 32, BS], F32, tag="ot",
                                       name=f"ot_{b}_{hc}_{q4}_{g}")
                        for nn in range(4):  # N=512 pieces (8 w each)
                            wg = q4 * 8 + g * 4 + nn     # global 8-w group
                            ps = pmm.tile([P, 8, BS], F32, tag="mm",
                                          name=f"psH_{b}_{hc}_{wg}")
                            nc.tensor.matmul(
                                ps[:], LIH[hc],
                                Ght[wg // 16][:, (wg % 16) * 8:(wg % 16) * 8 + 8, :],
                                start=True, stop=False)
                            # residual: += I^T @ x_bf (exact bf16 passthrough)
                            xslice = xr[:, (g * 4 + nn) * 8:(g * 4 + nn + 1) * 8, :]
                            nc.tensor.matmul(
                                ps[:], ident[:], xslice,
                                start=False, stop=True)
                            cp(ot[:, nn * 8:(nn + 1) * 8, :], ps[:])
                        nc.sync.dma_start(
                            out=out[b, hc * P:(hc + 1) * P,
                                    (q4 * 2 + g) * 32:(q4 * 2 + g + 1) * 32, :],
                            in_=ot[:])


def _prepare_in_maps(x, w1, b1, w2, b2):
    bf = ml_dtypes.bfloat16
    ffwd, lbw, liw, lih = _host_consts()
    x = np.asarray(x, dtype=np.float32)

    in_maps = []
    for n in range(NB):
        xs = np.ascontiguousarray(x[..., n * BS:(n + 1) * BS])
        w1n = np.asarray(w1[:, n], dtype=np.float64)   # (2,64,128)
        w2n = np.asarray(w2[:, n], dtype=np.float64)   # (2,128,64)
        b1n = np.asarray(b1[:, n], dtype=np.float32)   # (2,128)
        b2n = np.asarray(b2[:, n], dtype=np.float32)   # (2,64)
        m2 = [np.concatenate([w2n[0], w2n[1]], axis=1),
              np.concatenate([-w2n[1], w2n[0]], axis=1)]
        cb128 = np.concatenate(
            [ffwd[0], ffwd[1], lbw[0][0], lbw[0][1], lbw[1][0], lbw[1][1],
             m2[0], m2[1], lih[0], lih[1]], axis=1).astype(bf)
        m1 = [[w1n[0], -w1n[1]], [w1n[1], w1n[0]]]
        cb64 = np.concatenate(
            [m1[0][0], m1[0][1], m1[1][0], m1[1][1],
             liw[0][0][0], liw[0][0][1], liw[0][1][0], liw[0][1][1],
             liw[1][0][0], liw[1][0][1], liw[1][1][0], liw[1][1][1]],
            axis=1).astype(bf)
        cbias = np.stack([b1n[0], b1n[1],
                          np.concatenate([b2n[0], b2n[1]])], axis=1)
        in_maps.append({
            "xbf": xs.astype(bf),
            "cb128": cb128,
            "cb64": cb64,
            "cbias": np.ascontiguousarray(cbias, dtype=np.float32),
        })

    return in_maps


def kernel(x, w1, b1, w2, b2):
    global _CACHED_NC
    if _CACHED_NC is None:
        _CACHED_NC = _build_nc()
    nc = _CACHED_NC
    in_maps = _prepare_in_maps(x, w1, b1, w2, b2)
    res = run_bass_kernel_spmd(nc, in_maps, list(range(NB)))
    return np.concatenate([res.results[i]["out"] for i in range(NB)], axis=-1)
